# revision 73
# baseline (speedup 1.0000x reference)
"""LoFTR encoder layer (linear attention) on 8 Trainium2 NeuronCores.

Sharding: core c -> (n = c//2, L-half = c%2): 4096 query tokens per core.
With PAIR_SPLIT the source is also split between the two cores of a batch
(4096 rows each) and the tiny KV state [128, 2, 257] is AllReduce'd within
the pair, halving all phase-1 work; the collective's latency hides behind
the Q projections, which are emitted after it. All matmuls bf16 (fp32 PSUM).

Structure:
- ONE activation-table load (natural_log_exp_and_others): Exp/Ln/Copy/
  Identity/Relu all live in that set, so the ACT engine never reloads its
  function table (the default pass alternated tables, 47 reloads = ~60us).
- K|V projections fused into single N=512 matmuls; elu(k)+1 computed as
  min(exp(k),1) + relu(k) so Exp reads PSUM directly (ACT), the clamp runs
  on the otherwise-idle gpsimd engine, and one DVE op finishes the sum.
  Phase 1 is software-pipelined: KV-state matmuls trail the projections by
  2 half-groups so PE never waits on the elu chain.
- Z = 1/(Q.Ksum): Ksum is block-broadcast into a 128x128 lhsT (bcksum) so
  the denominator lands replicated across partitions in one matmul;
  reciprocal_approx_fast (DVE, ~18 bits, eps dropped as qk >= ~10); Z is
  applied to qe on gpsimd BEFORE the attention matmul.
- BD (block-diag KV state) folded into Wmerge on device (bdwm = BD^T @ Wm
  via a DVE 32x32 block-transpose + one matmul), so attention+merge is a
  single matmul from qz and the big per-tile Z multiply disappears.
- LayerNorms token-major (bn_stats); normalize via ACT Identity with
  per-partition scale/bias APs (LN1) and DVE tensor_scalar (LN2).
  ln1 gamma/beta folded into Wmlp1; ln2_g==1 / ln1_b==0 specialized at
  runtime; ln2_b + residual added on host; f16 output.
- Phase 2 software-pipelined: front (qk/Z/qz) at iteration i, mid
  (merge+LN1) at i-1, back (transpose/MLP1/MLP2/LN2/store) at i-4, so the
  back stage's lnm input is always 3 iterations old and PE never waits on
  the LN chain. PSUM plan exactly 8 banks (qk/lnT/h_ps share a 2-buf
  rotation via a bf16 bitcast view; the four 1-bank merge/MLP2 tiles
  rotate through 4 bufs; msg tiles are produced and consumed within one
  iteration so the deep pipeline costs no extra banks, only SBUF lnm/qz
  buffers). LN2/res split across DVE+ACT; res DMA in halves (tail drain).
"""

import numpy as np
import ml_dtypes

import concourse.bass as bass
import concourse.bacc as bacc
import concourse.tile as tile
from concourse import mybir
from concourse.bass_utils import run_bass_kernel_spmd

F32 = mybir.dt.float32
F16 = mybir.dt.float16
BF16 = mybir.dt.bfloat16
AF = mybir.ActivationFunctionType
ALU = mybir.AluOpType

D_MODEL = 256
NHEAD = 8
HEAD_DIM = 32
LN_EPS = 1e-7

ACT_TABLE_NL_EXP = 6  # natural_log_exp_and_others in act_info.json


def build_kernel(nc, TBLK, SBLK, skip_g2=True, skip_b1=True, pair_split=False):
    """Emit the per-core program. TBLK = query-token 128-blocks (32 full),
    SBLK = source-token 128-blocks (64 on-core; 32 when pair_split and the
    partial KV state is AllReduced between the two cores of a batch)."""
    C = D_MODEL
    NT = TBLK // 4   # q-tiles of 512 tokens
    if pair_split:
        SBLK = SBLK // 2
    NSG = SBLK // 4  # source groups of 512 rows

    # ---- DRAM I/O (per-core, host pre-shaped) ----
    x_pre = nc.dram_tensor("x_pre", [128, 2, TBLK * 128], BF16,
                           kind="ExternalInput").ap()
    s_pre = nc.dram_tensor("s_pre", [128, 2, SBLK * 128], BF16,
                           kind="ExternalInput").ap()
    wq_t = nc.dram_tensor("wq_t", [128, 2, 2, 128], BF16, kind="ExternalInput").ap()
    wkv_d = nc.dram_tensor("wkv_r", [128, 2, 512], BF16, kind="ExternalInput").ap()
    wm_d = nc.dram_tensor("wm_r", [128, 2, 256], BF16, kind="ExternalInput").ap()
    w1_t = nc.dram_tensor("w1_t", [128, 4, 4, 128], BF16, kind="ExternalInput").ap()
    w2_d = nc.dram_tensor("w2_r", [128, 4, 256], BF16, kind="ExternalInput").ap()
    b1p_d = nc.dram_tensor("b1p", [128, 4], F32, kind="ExternalInput").ap()
    ident_d = nc.dram_tensor("ident", [128, 128], BF16, kind="ExternalInput").ap()
    if not skip_g2:
        g2rep_d = nc.dram_tensor("g2rep", [128, 256], F32,
                                 kind="ExternalInput").ap()
    res_d = nc.dram_tensor("res", [128, TBLK, C], F16, kind="ExternalOutput").ap()

    from contextlib import ExitStack
    tc = nc.tc
    ctx = ExitStack()
    nc._pool_ctx = ctx

    # one act-table load; all ACT funcs below live in this set
    nc.scalar.add_instruction(mybir.InstLoadActFuncSet(
        name=nc.get_next_instruction_name(), ins=[], outs=[],
        act_func_set_id=ACT_TABLE_NL_EXP))

    consts = ctx.enter_context(tc.tile_pool(name="consts", bufs=1))
    persist = ctx.enter_context(tc.tile_pool(name="persist", bufs=1))
    work = ctx.enter_context(tc.tile_pool(name="work", bufs=2))
    psA_cm = tc.tile_pool(name="psA", bufs=1, space="PSUM")
    psA = psA_cm.__enter__()

    # ---- constants ----
    wq = consts.tile([128, 2, 2, 128], BF16, name="wq")
    wkv = consts.tile([128, 2, 512], BF16, name="wkv")
    wm = consts.tile([128, 2, 256], BF16, name="wm")
    w1 = consts.tile([128, 4, 4, 128], BF16, name="w1")
    w2 = consts.tile([128, 4, 256], BF16, name="w2")
    b1p = consts.tile([128, 4], F32, name="b1p")
    ident = consts.tile([128, 128], BF16, name="ident")
    eps_l = consts.tile([128, 1], F32, name="eps_l")
    ones_sb = consts.tile([128, 32], BF16, name="ones_sb")
    if not skip_g2:
        g2rep = consts.tile([128, 256], F32, name="g2rep")

    # ---- persistent activations ----
    xf = persist.tile([128, 2, TBLK * 128], BF16, name="xf")
    srcf = persist.tile([128, 2, SBLK * 128], BF16, name="srcf")
    qe = persist.tile([128, 2, TBLK * 128], BF16, name="qe")
    bd = persist.tile([128, 2, 128], BF16, name="bd")
    bdT = persist.tile([128, 2, 128], BF16, name="bdT")
    bcksum = persist.tile([128, 2, 128], BF16, name="bcksum")
    bdwm = persist.tile([128, 2, 256], BF16, name="bdwm")

    # ---- input DMAs (HWDGE), ordered so compute starts early ----
    half_x = TBLK * 64
    n_src_chunks = 2 if pair_split else 4
    q_s = SBLK * 128 // n_src_chunks
    nc.sync.dma_start(out=wq[:], in_=wq_t)
    nc.sync.dma_start(out=wkv[:], in_=wkv_d)
    qx = half_x // 2
    # first sub-chunks tiny (1 q-tile / 2 source half-groups) so the first
    # matmuls start as early as possible
    nc.sync.dma_start(out=xf[:, :, 0:512], in_=x_pre[:, :, 0:512])
    nc.sync.dma_start(out=srcf[:, :, 0:512], in_=s_pre[:, :, 0:512])
    nc.sync.dma_start(out=xf[:, :, 512:qx], in_=x_pre[:, :, 512:qx])
    nc.sync.dma_start(out=srcf[:, :, 512:q_s // 2],
                      in_=s_pre[:, :, 512:q_s // 2])
    nc.sync.dma_start(out=xf[:, :, qx:half_x], in_=x_pre[:, :, qx:half_x])
    nc.sync.dma_start(out=srcf[:, :, q_s // 2:q_s],
                      in_=s_pre[:, :, q_s // 2:q_s])
    for d, s in ((wm, wm_d), (w1, w1_t), (w2, w2_d), (b1p, b1p_d),
                 (ident, ident_d)):
        nc.sync.dma_start(out=d[:], in_=s)
    if not skip_g2:
        nc.sync.dma_start(out=g2rep[:], in_=g2rep_d)
    nc.sync.dma_start(out=xf[:, :, half_x:], in_=x_pre[:, :, half_x:])
    n_src_chunks = 2 if pair_split else 4
    for ch in range(1, n_src_chunks):
        sl = slice(q_s * ch, q_s * (ch + 1))
        nc.sync.dma_start(out=srcf[:, :, sl], in_=s_pre[:, :, sl])
    nc.vector.memset(eps_l, LN_EPS)
    nc.vector.memset(ones_sb, 1.0)

    # ================= phase 1: K/V+KV groups interleaved with Q =========
    # Software-pipelined: KV-state matmuls run 2 half-groups behind the
    # linear projections so PE never waits on the elu chain.
    kv_ps = psA.tile([128, 2, 512], F32, name="kv")   # 2 banks, accumulator
    NHG = 2 * NSG
    keve = {}

    def emit_kv_mms(h):
        ke_h, ve_h = keve.pop(h)
        for jj in range(2):
            for c in range(2):
                nc.tensor.matmul(
                    kv_ps[:, c, 0:257],
                    lhsT=ke_h[:, jj, 128 * c:128 * (c + 1)],
                    rhs=ve_h[:, jj, 0:257],
                    start=(h == 0 and jj == 0),
                    stop=(h == NHG - 1 and jj == 1))

    def emit_q_tile(t, alt=False):
        cols = slice(512 * t, 512 * (t + 1))
        # after the KV loop the "kvp" banks are free: alternate with "q"
        # so consecutive Q tiles pipeline instead of serializing. In
        # pair_split there is no "q" name (kvp has 3 bufs): all share kvp.
        if pair_split:
            q_ps = psA.tile([128, 2, 512], F32, name="kvp", bufs=3)
        else:
            nm = "kvp" if (alt and t % 2 == 1) else "q"
            q_ps = psA.tile([128, 2, 512], F32, name=nm,
                            bufs=2 if nm == "kvp" else 1)
        for m in range(2):
            for k in range(2):
                nc.tensor.matmul(q_ps[:, m, :], lhsT=wq[:, k, m, :],
                                 rhs=xf[:, k, cols], start=(k == 0),
                                 stop=(k == 1))
        qee = work.tile([128, 2, 512], BF16, name="qee")
        nc.scalar.activation(out=qee[:], in_=q_ps[:], func=AF.Exp, scale=1.0)
        qem = work.tile([128, 2, 512], BF16, name="qem")
        nc.gpsimd.tensor_scalar_min(qem[:], qee[:], 1.0)
        qr = work.tile([128, 2, 512], BF16, name="qr")
        nc.scalar.activation(out=qr[:], in_=q_ps[:], func=AF.Relu, scale=1.0)
        nc.gpsimd.tensor_tensor(out=qe[:, :, cols], in0=qr[:], in1=qem[:],
                                op=ALU.add)

    for hg in range(NHG + 2):
        if hg < NHG:
            kvp = psA.tile([128, 2, 512], F32, name="kvp",
                           bufs=3 if pair_split else 2)
            for jj in range(2):
                sc = slice(256 * hg + 128 * jj, 256 * hg + 128 * (jj + 1))
                for k in range(2):
                    nc.tensor.matmul(kvp[:, jj, :], lhsT=srcf[:, k, sc],
                                     rhs=wkv[:, k, :], start=(k == 0),
                                     stop=(k == 1))
            # elu(k)+1 = min(exp(k),1) + relu(k): Exp straight from PSUM,
            # clamp on the (otherwise idle) gpsimd engine
            kee = work.tile([128, 2, 256], BF16, name="kee")
            nc.scalar.activation(out=kee[:], in_=kvp[:, :, 0:256], func=AF.Exp,
                                 scale=1.0)
            kem = work.tile([128, 2, 256], BF16, name="kem")
            nc.gpsimd.tensor_scalar_min(kem[:], kee[:], 1.0)
            ke = work.tile([128, 2, 256], BF16, name="ke", bufs=3)
            nc.vector.scalar_tensor_tensor(
                out=ke[:], in0=kvp[:, :, 0:256], scalar=0.0, in1=kem[:],
                op0=ALU.max, op1=ALU.add)
            ve = work.tile([128, 2, 264], BF16, name="ve", bufs=3)
            if hg % 2 == 0:
                nc.scalar.activation(out=ve[:, :, 0:256],
                                     in_=kvp[:, :, 256:512], func=AF.Copy)
            else:
                nc.vector.tensor_copy(ve[:, :, 0:256], kvp[:, :, 256:512])
            nc.vector.memset(ve[:, :, 256:257], 1.0)
            keve[hg] = (ke, ve)
        if hg >= 2:
            emit_kv_mms(hg - 2)
        if not pair_split:
            qstride = NHG // NT
            if hg % qstride == 1 and hg // qstride < NT:
                emit_q_tile(hg // qstride)

    # ---- pair AllReduce of the partial KV state (pair_split only) ----
    # Issue the collective first, then run all Q-projection work so the
    # collective's latency hides behind it.
    if pair_split:
        kvsb = work.tile([128, 2, 257], F32, name="kvsb")
        nc.scalar.activation(out=kvsb[:], in_=kv_ps[:, :, 0:257], func=AF.Copy)
        kvpart_d = nc.dram_tensor("kvpart", [128, 2, 257], F32).ap()
        kvred_d = nc.dram_tensor("kvred", [128, 2, 257], F32).ap()
        nc.sync.dma_start(out=kvpart_d, in_=kvsb[:])
        nc.gpsimd.collective_compute(
            "AllReduce", ALU.add,
            replica_groups=[[0, 1], [2, 3], [4, 5], [6, 7]],
            ins=[kvpart_d], outs=[kvred_d])
        for t in range(NT):
            emit_q_tile(t, alt=True)
        kv_full = work.tile([128, 2, 257], F32, name="kv_full")
        nc.sync.dma_start(out=kv_full[:], in_=kvred_d)
    else:
        kv_full = kv_ps

    # ---- BD / bcksum / bdwm extraction ----
    nc.vector.memset(bd, 0.0)
    nc.vector.memset(bcksum, 0.0)
    ksum_sb = work.tile([128, 2, 1], F32, name="ksum_sb")
    nc.scalar.activation(out=ksum_sb[:], in_=kv_full[:, :, 256:257],
                         func=AF.Copy)
    for c in range(2):
        for r in range(4):
            h = 4 * c + r
            rows = slice(32 * r, 32 * (r + 1))
            nc.scalar.activation(out=bd[rows, c, rows],
                                 in_=kv_full[rows, c, 32 * h:32 * h + 32],
                                 func=AF.Copy)
            nc.vector.tensor_scalar_mul(
                bcksum[rows, c, rows], ones_sb[rows, 0:32],
                ksum_sb[rows, c, :])
    nc.vector.transpose(out=bdT[:], in_=bd[:])  # per-32x32-block transpose
    bdwm_ps = psA.tile([128, 2, 512], F32,
                       name="kvp" if pair_split else "q",
                       bufs=3 if pair_split else 1)
    for c in range(2):
        nc.tensor.matmul(bdwm_ps[:, c, 0:256], lhsT=bdT[:, c, :],
                         rhs=wm[:, c, :], start=True, stop=True)
    nc.scalar.activation(out=bdwm[:], in_=bdwm_ps[:, :, 0:256], func=AF.Copy)

    psA_cm.__exit__(None, None, None)
    psB_cm = tc.tile_pool(name="psB", bufs=1, space="PSUM")
    psB = psB_cm.__enter__()
    ctx.callback(lambda: psB_cm.__exit__(None, None, None))

    # ================= phase 2: software-pipelined over q-tiles ==========
    qz_of, lnm_of = {}, {}
    for it in range(NT + 4):
        t1, t2, t3 = it, it - 1, it - 4
        if t1 < NT:
            # ---- front: qk -> Z -> qz ----
            cols = slice(512 * t1, 512 * (t1 + 1))
            qk_ps = psB.tile([128, 2, 512], F32, name="qk", bufs=2)
            for c in range(2):
                nc.tensor.matmul(qk_ps[:, c, :], lhsT=bcksum[:, c, :],
                                 rhs=qe[:, c, cols], start=True, stop=True)
            zrep = work.tile([128, 2, 512], F32, name="zrep")
            nc.vector.reciprocal_approx_fast(out=zrep[:], in_=qk_ps[:])
            qz = work.tile([128, 2, 512], BF16, name="qz")
            nc.gpsimd.tensor_mul(qz[:], qe[:, :, cols], zrep[:])
            qz_of[t1] = qz
        if 0 <= t2 < NT:
            # ---- mid: merge (token-major) + LN1 ----
            msga = psB.tile([128, 2, 256], F32, name="pj", bufs=4)
            msgb = psB.tile([128, 2, 256], F32, name="pj", bufs=4)
            qz2 = qz_of.pop(t2)
            for j in range(4):
                # one accumulation group per PSUM bank (two j's per bank)
                dst = msga[:, j, :] if j < 2 else msgb[:, j - 2, :]
                first, last = j % 2 == 0, j % 2 == 1
                for c in range(2):
                    nc.tensor.matmul(dst, lhsT=qz2[:, c, 128 * j:128 * (j + 1)],
                                     rhs=bdwm[:, c, :],
                                     start=(first and c == 0),
                                     stop=(last and c == 1))
            st1 = work.tile([128, 4, 6], F32, name="st1")
            mv1 = work.tile([128, 4, 2], F32, name="mv1")
            for j in range(4):
                src = msga[:, j, :] if j < 2 else msgb[:, j - 2, :]
                nc.vector.bn_stats(out=st1[:, j, :], in_=src)
                nc.vector.bn_aggr(out=mv1[:, j, :], in_=st1[:, j, :])
            lnv1 = work.tile([128, 4], F32, name="lnv1")
            nc.scalar.activation(out=lnv1[:], in_=mv1[:, :, 1], func=AF.Ln,
                                 bias=eps_l[:], scale=1.0)
            rstd1 = work.tile([128, 4], F32, name="rstd1")
            nc.scalar.activation(out=rstd1[:], in_=lnv1[:], func=AF.Exp,
                                 scale=-0.5)
            nmr1 = work.tile([128, 4], F32, name="nmr1")
            nc.vector.scalar_tensor_tensor(
                out=nmr1[:], in0=mv1[:, :, 0], scalar=-1.0, in1=rstd1[:],
                op0=ALU.mult, op1=ALU.mult)
            lnm = work.tile([128, 4, 256], BF16, name="lnm", bufs=4)
            for j in range(4):
                src = msga[:, j, :] if j < 2 else msgb[:, j - 2, :]
                nc.scalar.activation(out=lnm[:, j, :], in_=src,
                                     func=AF.Identity,
                                     bias=nmr1[:, j:j + 1],
                                     scale=rstd1[:, j:j + 1])
            lnm_of[t2] = lnm
        if 0 <= t3:
            # ---- back: transpose, MLP1, MLP2, LN2, out ----
            cols = slice(512 * t3, 512 * (t3 + 1))
            lnm2 = lnm_of.pop(t3)
            lnT_f = psB.tile([128, 2, 512], F32, name="qk", bufs=2)
            lnT_ps = lnT_f[:, :, 0:256].bitcast(BF16)  # bf16 view, 1 bank
            for j in range(4):
                for c in range(2):
                    nc.tensor.transpose(out=lnT_ps[:, c, 128 * j:128 * (j + 1)],
                                        in_=lnm2[:, j, 128 * c:128 * (c + 1)],
                                        identity=ident[:])
            lnmT = work.tile([128, 2, 512], BF16, name="lnmT")
            nc.scalar.activation(out=lnmT[:], in_=lnT_ps[:], func=AF.Copy)
            h_sb = work.tile([128, 4, 512], BF16, name="h_sb")
            for mp in range(2):  # m-pairs; one accumulation group per bank
                h_ps = psB.tile([128, 2, 512], F32, name="qk", bufs=2)
                for mi in range(2):
                    m = 2 * mp + mi
                    for k in range(4):
                        rhs = xf[:, k, cols] if k < 2 else lnmT[:, k - 2, :]
                        nc.tensor.matmul(h_ps[:, mi, :], lhsT=w1[:, k, m, :],
                                         rhs=rhs, start=(k == 0),
                                         stop=(k == 3))
                if skip_b1:
                    nc.scalar.activation(
                        out=h_sb[:, 2 * mp:2 * (mp + 1), :], in_=h_ps[:],
                        func=AF.Relu, scale=1.0)
                else:
                    for mi in range(2):
                        m = 2 * mp + mi
                        nc.scalar.activation(out=h_sb[:, m, :],
                                             in_=h_ps[:, mi, :],
                                             func=AF.Relu,
                                             bias=b1p[:, m:m + 1], scale=1.0)
            m2a = psB.tile([128, 2, 256], F32, name="pj", bufs=4)
            m2b = psB.tile([128, 2, 256], F32, name="pj", bufs=4)
            for k in range(4):
                for j in range(4):
                    # one accumulation group per bank (two j's per bank)
                    dst = m2a[:, j, :] if j < 2 else m2b[:, j - 2, :]
                    nc.tensor.matmul(dst,
                                     lhsT=h_sb[:, k, 128 * j:128 * (j + 1)],
                                     rhs=w2[:, k, :],
                                     start=(k == 0 and j % 2 == 0),
                                     stop=(k == 3 and j % 2 == 1))
            st2 = work.tile([128, 4, 6], F32, name="st2")
            mv2 = work.tile([128, 4, 2], F32, name="mv2")
            for j in range(4):
                src = m2a[:, j, :] if j < 2 else m2b[:, j - 2, :]
                nc.vector.bn_stats(out=st2[:, j, :], in_=src)
                nc.vector.bn_aggr(out=mv2[:, j, :], in_=st2[:, j, :])
            lnv2 = work.tile([128, 4], F32, name="lnv2")
            nc.scalar.activation(out=lnv2[:], in_=mv2[:, :, 1], func=AF.Ln,
                                 bias=eps_l[:], scale=1.0)
            rstd2 = work.tile([128, 4], F32, name="rstd2")
            nc.scalar.activation(out=rstd2[:], in_=lnv2[:], func=AF.Exp,
                                 scale=-0.5)
            nmr2 = work.tile([128, 4], F32, name="nmr2")
            nc.vector.scalar_tensor_tensor(
                out=nmr2[:], in0=mv2[:, :, 0], scalar=-1.0, in1=rstd2[:],
                op0=ALU.mult, op1=ALU.mult)
            res_sb = work.tile([128, 4, 256], F16, name="res_sb")
            if skip_g2:
                for j in range(4):
                    src = m2a[:, j, :] if j < 2 else m2b[:, j - 2, :]
                    if j < 2:
                        nc.vector.tensor_scalar(
                            out=res_sb[:, j, :], in0=src,
                            scalar1=mv2[:, j, 0:1], scalar2=rstd2[:, j:j + 1],
                            op0=ALU.subtract, op1=ALU.mult)
                    else:
                        nc.scalar.activation(out=res_sb[:, j, :], in_=src,
                                             func=AF.Identity,
                                             bias=nmr2[:, j:j + 1],
                                             scale=rstd2[:, j:j + 1])
            else:
                lng = work.tile([128, 4, 256], F32, name="lng")
                for j in range(4):
                    src = m2a[:, j, :] if j < 2 else m2b[:, j - 2, :]
                    nc.vector.tensor_scalar(
                        out=lng[:, j, :], in0=src,
                        scalar1=mv2[:, j, 0:1], scalar2=rstd2[:, j:j + 1],
                        op0=ALU.subtract, op1=ALU.mult)
                for j in range(4):
                    nc.vector.tensor_mul(res_sb[:, j, :], lng[:, j, :],
                                         g2rep[:])
            nc.sync.dma_start(out=res_d[:, 4 * t3:4 * t3 + 2, :],
                              in_=res_sb[:, 0:2, :])
            nc.sync.dma_start(out=res_d[:, 4 * t3 + 2:4 * t3 + 4, :],
                              in_=res_sb[:, 2:4, :])


def _prep_host(inputs, TBLK, SBLK):
    """Shared host-side prep. Returns (const_map, per-core fn, skip_g2)."""
    bf = ml_dtypes.bfloat16
    Wq, Wk, Wv = inputs["Wq"], inputs["Wk"], inputs["Wv"]
    Wm, W1, W2 = inputs["Wmerge"], inputs["Wmlp1"], inputs["Wmlp2"]
    g1, b1 = inputs["ln1_g"], inputs["ln1_b"]
    g2 = np.asarray(inputs["ln2_g"], np.float32)
    skip_g2 = bool(np.all(g2 == 1.0))
    skip_b1 = bool(np.all(np.asarray(inputs["ln1_b"]) == 0.0))
    # fold ln1 gamma/beta into W1: h = relu(cat[x, g1*n + b1] @ W1.T)
    W1s = np.asarray(W1).copy()
    W1s[:, 256:] = W1[:, 256:] * np.asarray(g1)[None, :]
    b1p = (np.asarray(W1)[:, 256:] @ np.asarray(b1)).astype(np.float32)
    WkT = np.ascontiguousarray(np.asarray(Wk).T.reshape(2, 128, 256)
                               .transpose(1, 0, 2))
    WvT = np.ascontiguousarray(np.asarray(Wv).T.reshape(2, 128, 256)
                               .transpose(1, 0, 2))
    const = {
        "wq_t": np.ascontiguousarray(
            np.asarray(Wq).T.reshape(2, 128, 2, 128).transpose(1, 0, 2, 3)
        ).astype(bf),
        "wkv_r": np.concatenate([WkT, WvT], axis=2).astype(bf),
        "wm_r": np.ascontiguousarray(np.asarray(Wm).T.reshape(2, 128, 256)
                                     .transpose(1, 0, 2)).astype(bf),
        "w1_t": np.ascontiguousarray(
            W1s.T.reshape(4, 128, 4, 128).transpose(1, 0, 2, 3)).astype(bf),
        "w2_r": np.ascontiguousarray(np.asarray(W2).T.reshape(4, 128, 256)
                                     .transpose(1, 0, 2)).astype(bf),
        "b1p": np.ascontiguousarray(b1p.reshape(4, 128).T).astype(np.float32),
        "ident": np.eye(128, dtype=bf),
    }
    if not skip_g2:
        const["g2rep"] = np.broadcast_to(g2, (128, 256)).copy()

    def blocks(a, nblk):  # token-major [T, C] -> feature-major [128, 2, T]
        del nblk
        T = a.shape[0]
        return np.ascontiguousarray(
            a.T.reshape(2, 128, T).transpose(1, 0, 2)).astype(bf)

    return const, blocks, skip_g2, skip_b1


TRACE = False        # set by test harness for NTFF profiling
LAST_RESULT = None   # BassKernelResults of the last kernel() call
PAIR_SPLIT = True    # shard S across the two cores of a batch + AllReduce KV


def kernel(**inputs):
    global LAST_RESULT
    TBLK, SBLK = 32, 64
    N, L, C = inputs["x"].shape
    x = np.asarray(inputs["x"], np.float32)
    source = np.asarray(inputs["source"], np.float32)
    const, blocks, skip_g2, skip_b1 = _prep_host(inputs, TBLK, SBLK)

    nc = bacc.Bacc("TRN2", target_bir_lowering=False, debug=False, num_devices=8)
    with tile.TileContext(nc) as tc:
        nc.tc = tc
        build_kernel(nc, TBLK, SBLK, skip_g2=skip_g2, skip_b1=skip_b1,
                     pair_split=PAIR_SPLIT)
        nc._pool_ctx.close()
    nc.compile()

    in_maps = []
    for c in range(8):
        n, half = c // 2, c % 2
        xs = x[n, 4096 * half:4096 * (half + 1)]
        src = (source[n, 4096 * half:4096 * (half + 1)] if PAIR_SPLIT
               else source[n])
        in_maps.append({**const,
                        "x_pre": blocks(xs, TBLK),
                        "s_pre": blocks(src, SBLK)})
    LAST_RESULT = run_bass_kernel_spmd(nc, in_maps, core_ids=list(range(8)),
                                       trace=TRACE)
    res = LAST_RESULT.results

    out = np.empty((N, L, C), np.float32)
    b2 = np.asarray(inputs["ln2_b"], np.float32)
    for c in range(8):
        n, half = c // 2, c % 2
        r = np.asarray(res[c]["res"], np.float32).transpose(1, 0, 2)
        r = r.reshape(4096, C)
        out[n, 4096 * half:4096 * (half + 1)] = (
            x[n, 4096 * half:4096 * (half + 1)] + b2[None, :] + r)
    return out
